# revision 1
# baseline (speedup 1.0000x reference)
"""Trainium2 kernel for FFT-based converged inhibition along the channel axis.

The reference computes y = IFFT(FFT(x, axis=C) / FFT(delta - k_padded)).real,
i.e. a circular convolution of each channel fiber with the fixed length-C
kernel g = IFFT(1/FFT(delta - k)).  That is a circulant matmul Y = G @ X with
G[m, c] = g[(m - c) mod C], applied independently at every (batch, h, w).

Device strategy (8 NeuronCores, data-parallel over batch):
  - each core gets 2 of the 16 batches: X_core [2, 512, 3136]
  - Y[b] = G @ X[b] as TensorE matmuls: lhsT = G^T tiles [128k, 128m],
    rhs = X k-tiles [128, 448], accumulate over k in PSUM.
  - g decays below ~1e-15 beyond +/-128 channels for this filter, so the
    k-tile at circular block distance 2 is skipped (3 of 4 k-tiles per
    output tile); verified numerically at build time, falls back to dense.
  - operands are fed as float32r (full-rate fp32 matmul mode on TRN2).
"""

import numpy as np

import concourse.bass as bass
import concourse.tile as tile
from concourse import bacc, mybir
from concourse.bass_utils import run_bass_kernel_spmd

N_CORES = 8
C = 512  # channels (FFT axis)
KT = C // 128  # 4 k/m tiles of 128 channels
FCH = 448  # free-dim chunk (fits one PSUM bank, uniform: 3136 = 7*448)

_CACHE = {}


def _build_program(n_batch_per_core: int, hw: int, band3: bool):
    """Per-core SPMD program: y[b] = G @ x[b] for n_batch_per_core batches."""
    nfc = hw // FCH
    assert nfc * FCH == hw
    nc = bacc.Bacc(
        "TRN2", target_bir_lowering=False, debug=False, enable_asserts=False
    )
    # band3: G^T ships only the 3 used m-blocks per k-tile (j = (m-kt+1)%KT)
    gw = 3 * 128 if band3 else C
    x_d = nc.dram_tensor(
        "x", [n_batch_per_core, C, hw], mybir.dt.float32r, kind="ExternalInput"
    ).ap()
    gt_d = nc.dram_tensor(
        "gt", [KT, 128, gw], mybir.dt.float32r, kind="ExternalInput"
    ).ap()
    y_d = nc.dram_tensor(
        "y", [n_batch_per_core, C, hw], mybir.dt.float32, kind="ExternalOutput"
    ).ap()

    # first output tile m=0 needs k-tiles {3, 0, 1}
    first_ks = [3, 0, 1] if band3 else [0, 1, 2, 3]
    nfa = (nfc + 1) // 2  # chunks in first input half (4 of 7)
    ca = nfa * FCH  # split column (1792)

    # DMA plan: HWDGE lanes are assigned round-robin in scheduled order; keep
    # the order such that no input dispatch ever waits on an output-occupied
    # lane.  rhs[(b, kt, f)] = (tile, col_offset) for matmul rhs slicing.
    rhs = {}

    with tile.TileContext(nc) as tc:
        with (
            tc.tile_pool(name="gt", bufs=1) as gt_pool,
            tc.tile_pool(name="x", bufs=1) as x_pool,
            tc.tile_pool(name="ps", bufs=8, space="PSUM") as ps_pool,
            tc.tile_pool(name="out", bufs=4) as out_pool,
        ):
            # 1 DMA: all of (packed) G^T  [128, kt, m-block]
            gt_sb = gt_pool.tile([128, KT, gw], mybir.dt.float32r, tag="gt")
            nc.sync.dma_start(gt_sb[:], gt_d.rearrange("kt p m -> p kt m"))

            def w_slice(kt, m):
                j = (m - kt + 1) % KT if band3 else m
                return gt_sb[:, kt, 128 * j : 128 * (j + 1)]

            def in_dma(b, kt, c0, c1, tag):
                t = x_pool.tile([128, c1 - c0], mybir.dt.float32r, tag=tag)
                nc.sync.dma_start(t[:], x_d[b, 128 * kt : 128 * (kt + 1), c0:c1])
                for f in range(c0 // FCH, c1 // FCH):
                    rhs[(b, kt, f)] = (t[:], f * FCH - c0)
                return t

            # Inputs in rounds of <=8 DMAs.  HWDGE lanes are assigned
            # round-robin over 8 in scheduled order and each lane is FIFO, so
            # round r+1 transfers only start as round r completes: sections
            # complete in consumption order and all inputs run before outputs.
            kseq = first_ks + [kt for kt in range(KT) if kt not in first_ks]
            # round 1 (with gt): b0 half A, f0 chunks first for early PE start
            for kt in first_ks:
                tt = x_pool.tile([128, FCH], mybir.dt.float32r, tag=f"xf{kt}")
                nc.sync.dma_start(tt[:], x_d[0, 128 * kt : 128 * (kt + 1), :FCH])
                rhs[(0, kt, 0)] = (tt[:], 0)
            for kt in first_ks:
                tr = x_pool.tile([128, ca - FCH], mybir.dt.float32r, tag=f"xr{kt}")
                nc.sync.dma_start(tr[:], x_d[0, 128 * kt : 128 * (kt + 1), FCH:ca])
                for f in range(1, nfa):
                    rhs[(0, kt, f)] = (tr[:], (f - 1) * FCH)
            for kt in range(KT):
                if kt not in first_ks:
                    in_dma(0, kt, 0, ca, f"xa{kt}")
            # round 2: b0 half B then b1 half A; round 3: b1 half B
            for kt in kseq:
                in_dma(0, kt, ca, hw, f"xb{kt}")
            for b in range(1, n_batch_per_core):
                for kt in kseq:
                    in_dma(b, kt, 0, ca, f"xc{b}_{kt}")
                for kt in kseq:
                    in_dma(b, kt, ca, hw, f"xd{b}_{kt}")

            # process each batch's half-A columns (arrive first) across all m,
            # then half-B, so PE never stalls on late input halves
            for b in range(n_batch_per_core):
                for half, fr in enumerate([range(nfa), range(nfa, nfc)]):
                    c0 = 0 if half == 0 else ca
                    c1 = ca if half == 0 else hw
                    for m in range(KT):
                        if band3:
                            ks = [(m + KT - 1) % KT, m, (m + 1) % KT]
                        else:
                            ks = list(range(KT))
                        o = out_pool.tile(
                            [128, c1 - c0], mybir.dt.float32, tag=f"out{half}"
                        )
                        for f in fr:
                            ps = ps_pool.tile(
                                [128, FCH],
                                mybir.dt.float32,
                                tag="ps",
                                name=f"ps{b}_{m}_{f}",
                            )
                            for ki, kt in enumerate(ks):
                                t, off = rhs[(b, kt, f)]
                                nc.tensor.matmul(
                                    ps[:],
                                    w_slice(kt, m),
                                    t[:, off : off + FCH],
                                    start=(ki == 0),
                                    stop=(ki == len(ks) - 1),
                                )
                            dst = o[:, FCH * f - c0 : FCH * (f + 1) - c0]
                            if f % 2 == 0:
                                nc.vector.tensor_copy(dst, ps[:])
                            else:
                                nc.scalar.mul(dst, ps[:], 1.0)
                        nc.scalar.dma_start(
                            y_d[b, 128 * m : 128 * (m + 1), c0:c1], o[:]
                        )

    # Hoist the no-wait round-1 input DMA dispatches into the pre-barrier
    # main block: transfers then start while the other engines are still in
    # the kernel-entry barrier (~5us earlier).  Their lane-sem updates are
    # position-independent and walrus emits SET_ORDERING_MODE at the head of
    # the engine binary regardless of block placement.
    try:
        main_blk = nc.main_func.blocks[0]
        sp = mybir.EngineType.SP
        moved = None
        for blk in nc.main_func.blocks[1:]:
            cand = [
                i
                for i in blk.instructions
                if i.engine == sp
                and isinstance(i, mybir.InstDMACopy)
                and not (i.sync_info and i.sync_info.on_wait)
            ]
            if cand:
                moved = cand[:8]
                for i in moved:
                    blk.instructions.remove(i)
                break
        if moved:
            pos = next(
                idx
                for idx, i in enumerate(main_blk.instructions)
                if i.engine == sp and isinstance(i, mybir.InstDrain)
            )
            main_blk.instructions[pos:pos] = moved
    except Exception:
        pass

    # Strip the unused const-tile memsets Bass emits in its preamble: they
    # pull the gpsimd ucode library load into the critical entry barrier
    # (~2-8us of NEFF time) and nothing in this kernel reads them.
    for blk in nc.main_func.blocks:
        blk.instructions[:] = [
            inst
            for inst in blk.instructions
            if not (
                isinstance(inst, mybir.InstMemset)
                and inst.outs
                and "const-" in str(inst.outs[0])
            )
        ]
    nc.compile()
    return nc


def _circulant_gt(inhibition_filter: np.ndarray, c: int):
    """g = IFFT(1/FFT(delta - pad_roll(k))) in float64; returns (G^T, band3_ok)."""
    scope = inhibition_filter.shape[0]
    k = np.zeros(c, np.float64)
    k[:scope] = inhibition_filter.astype(np.float64)
    k = np.roll(k, -(scope // 2))
    delta = np.zeros(c, np.float64)
    delta[0] = 1.0
    g = np.fft.ifft(1.0 / np.fft.fft(delta - k)).real
    idx = (np.arange(c)[:, None] - np.arange(c)[None, :]) % c  # G[m, cc] = g[m-cc]
    G = g[idx]
    # band check: can the k-tile at circular block distance 2 be skipped?
    dist = np.minimum(np.arange(c), c - np.arange(c))
    tail = np.abs(g[dist > 128]).max() if (dist > 128).any() else 0.0
    band3_ok = tail <= 1e-9 * np.abs(g).max()
    return np.ascontiguousarray(G.T, dtype=np.float32), band3_ok


def _reset_device():
    """Recover a wedged NeuronCore (NRT_EXEC_UNIT_UNRECOVERABLE) via axon."""
    try:
        import ctypes

        import jax

        jax.devices()
        lib = ctypes.CDLL("/opt/axon/libaxon_pjrt.so")
        if hasattr(lib, "axon_reset"):
            lib.axon_reset.restype = ctypes.c_int64
            lib.axon_reset()
    except Exception:
        pass


def kernel(activations: np.ndarray, inhibition_filter: np.ndarray) -> np.ndarray:
    return _run(activations, inhibition_filter, trace=False)[0]


def _run(activations, inhibition_filter, trace=False):
    activations = np.ascontiguousarray(activations, dtype=np.float32)
    n, c, h, w_ = activations.shape
    assert c == C and n % N_CORES == 0
    hw = h * w_
    npc = n // N_CORES

    gt, band3 = _circulant_gt(np.asarray(inhibition_filter, np.float32), c)
    gt = gt.reshape(KT, 128, C)
    if band3:
        gtp = np.empty((KT, 128, 3 * 128), np.float32)
        for kt in range(KT):
            for j in range(3):
                m = (kt - 1 + j) % KT
                gtp[kt, :, 128 * j : 128 * (j + 1)] = gt[kt, :, 128 * m : 128 * (m + 1)]
        gt = np.ascontiguousarray(gtp)

    key = (npc, hw, band3)
    if key not in _CACHE:
        _CACHE[key] = _build_program(npc, hw, band3)
    nc = _CACHE[key]

    xs = activations.reshape(N_CORES, npc, C, hw)
    in_maps = [{"x": xs[i], "gt": gt} for i in range(N_CORES)]
    try:
        res = run_bass_kernel_spmd(nc, in_maps, list(range(N_CORES)), trace=trace)
    except Exception:
        _reset_device()
        res = run_bass_kernel_spmd(nc, in_maps, list(range(N_CORES)), trace=trace)
    y = np.stack([res.results[i]["y"] for i in range(N_CORES)])
    y = y.reshape(n, c, h, w_).astype(np.float32, copy=False)
    return y, res



# revision 3
# speedup vs baseline: 1.9292x; 1.9292x over previous
"""Trainium2 kernel for FFT-based converged inhibition along the channel axis.

The reference computes y = IFFT(FFT(x, axis=C) / FFT(delta - k_padded)).real,
i.e. a circular convolution of each channel fiber with the fixed length-C
kernel g = IFFT(1/FFT(delta - k)).  Writing h = g - delta, the output is
y = x + h (*) x where the correction h (*) x is SMALL (||h||_2 ~ 0.14 for
this damping) and h decays fast away from lag 0.

Device strategy (8 NeuronCores, data-parallel over batch):
  - the device computes ONLY the correction c = h (*) x in fp8 (float8e3,
    4 mantissa bits); the host adds y = x + c in fp32.  This cuts HBM
    traffic per element from 8 B (fp32 in+out) to ~2.3 B, and the kernel
    is DMA-roofline bound.
  - channel axis split into NW=5 output windows of M=104; window w reads
    input rows [104w-12, 104w+115] (128 rows incl +-12 halo, mod C) so a
    single K=128 matmul per (window, column chunk) produces 104 output
    channels with the full h restricted to the window (only window-edge
    outputs see one-sided tap truncation; measured rel-err ~7e-3 vs the
    2e-2 budget on white-noise activations).
  - the window weight matrix lhsT[kr, i] = h[i + 12 - kr] is the same for
    every window -> one [128, 104] stationary tile, one LDWEIGHTS total.
  - scales: x is sent as e3m4(x * SX), weights as e3m4(h * SW), PSUM holds
    SX*SW*c, and the PSUM->SBUF copy applies SC/(SX*SW) and casts to e3m4;
    the host divides by SC.  All scales are powers of two chosen at run
    time from max|x| and the h norms, so they are exact.
"""

import numpy as np
import ml_dtypes

import concourse.bass as bass
import concourse.tile as tile
from concourse import bacc, mybir
from concourse.bass_utils import run_bass_kernel_spmd

N_CORES = 8
C = 512          # channels (FFT axis)
NW = 5           # output windows along C
M = 104          # output channels per window (NW * M = 520 >= C)
T = 12           # one-sided halo: window w reads rows [M*w - T, M*w - T + 127]
WIN = 128        # input rows per window
FCH = 448        # matmul moving free-dim chunk (3136 = 7 * 448, one PSUM bank)

_CACHE = {}

F8 = ml_dtypes.float8_e3m4  # TRN FP8_EXP3 (e3m4), max +-31


def _build_program(npc: int, hw: int, out_scale: float):
    """Per-core SPMD program: c[w] = (h-window) @ x[w] for NW windows."""
    nfc = hw // FCH
    assert nfc * FCH == hw
    nb = NW * npc  # input blocks (window, batch)
    nc = bacc.Bacc(
        "TRN2", target_bir_lowering=False, debug=False, enable_asserts=False
    )
    x_d = nc.dram_tensor(
        "x", [128, nb * hw], mybir.dt.float8e3, kind="ExternalInput"
    ).ap()
    w_d = nc.dram_tensor(
        "wq", [128, M], mybir.dt.float8e3, kind="ExternalInput"
    ).ap()
    y_d = nc.dram_tensor(
        "y", [NW, M, npc * hw], mybir.dt.float8e3, kind="ExternalOutput"
    ).ap()

    with tile.TileContext(nc) as tc:
        with (
            tc.tile_pool(name="wq", bufs=1) as w_pool,
            tc.tile_pool(name="x", bufs=1) as x_pool,
            tc.tile_pool(name="ps", bufs=8, space="PSUM") as ps_pool,
            tc.tile_pool(name="out", bufs=1) as out_pool,
        ):
            w_sb = w_pool.tile([128, M], mybir.dt.float8e3, tag="wq")
            nc.sync.dma_start(w_sb[:], w_d)

            # inputs: one DMA per (window, batch) block, in consumption order
            xt = []
            for j in range(nb):
                t = x_pool.tile([128, hw], mybir.dt.float8e3, tag=f"x{j}")
                nc.sync.dma_start(t[:], x_d[:, j * hw : (j + 1) * hw])
                xt.append(t)

            for w in range(NW):
                o = out_pool.tile([M, npc * hw], mybir.dt.float8e3, tag=f"o{w}")
                for b in range(npc):
                    for f in range(nfc):
                        ps = ps_pool.tile(
                            [M, FCH], mybir.dt.float32, tag="ps",
                            name=f"ps{w}_{b}_{f}",
                        )
                        nc.tensor.matmul(
                            ps[:],
                            w_sb[:],
                            xt[w * npc + b][:, f * FCH : (f + 1) * FCH],
                            start=True,
                            stop=True,
                        )
                        dst = o[:, b * hw + f * FCH : b * hw + (f + 1) * FCH]
                        if f % 2 == 0:
                            nc.vector.tensor_scalar_mul(dst, ps[:], out_scale)
                        else:
                            nc.scalar.mul(dst, ps[:], out_scale)
                nc.scalar.dma_start(y_d[w], o[:])

    # Hoist no-wait input DMA dispatches into the pre-barrier main block so
    # transfers start while other engines are still in the entry barrier.
    try:
        main_blk = nc.main_func.blocks[0]
        sp = mybir.EngineType.SP
        moved = None
        for blk in nc.main_func.blocks[1:]:
            cand = [
                i
                for i in blk.instructions
                if i.engine == sp
                and isinstance(i, mybir.InstDMACopy)
                and not (i.sync_info and i.sync_info.on_wait)
            ]
            if cand:
                moved = cand[:8]
                for i in moved:
                    blk.instructions.remove(i)
                break
        if moved:
            pos = next(
                idx
                for idx, i in enumerate(main_blk.instructions)
                if i.engine == sp and isinstance(i, mybir.InstDrain)
            )
            main_blk.instructions[pos:pos] = moved
    except Exception:
        pass

    # Strip unused const-tile memsets from the preamble (they pull the gpsimd
    # ucode load into the critical entry barrier).
    for blk in nc.main_func.blocks:
        blk.instructions[:] = [
            inst
            for inst in blk.instructions
            if not (
                isinstance(inst, mybir.InstMemset)
                and inst.outs
                and "const-" in str(inst.outs[0])
            )
        ]
    nc.compile()
    return nc


def _inv_kernel(inhibition_filter: np.ndarray, c: int):
    """h = IFFT(1/FFT(delta - pad_roll(k))) - delta in float64."""
    scope = inhibition_filter.shape[0]
    k = np.zeros(c, np.float64)
    k[:scope] = inhibition_filter.astype(np.float64)
    k = np.roll(k, -(scope // 2))
    delta = np.zeros(c, np.float64)
    delta[0] = 1.0
    g = np.fft.ifft(1.0 / np.fft.fft(delta - k)).real
    return g - delta, delta - k


def _pow2(v: float) -> float:
    return float(2.0 ** np.floor(np.log2(v)))


def _reset_device():
    """Recover a wedged NeuronCore via axon."""
    try:
        import ctypes

        import jax

        jax.devices()
        lib = ctypes.CDLL("/opt/axon/libaxon_pjrt.so")
        if hasattr(lib, "axon_reset"):
            lib.axon_reset.restype = ctypes.c_int64
            lib.axon_reset()
    except Exception:
        pass


def kernel(activations: np.ndarray, inhibition_filter: np.ndarray) -> np.ndarray:
    return _run(activations, inhibition_filter, trace=False)[0]


def _run(activations, inhibition_filter, trace=False):
    x = np.ascontiguousarray(activations, dtype=np.float32)
    n, c, hgt, wid = x.shape
    hw = hgt * wid
    npc = n // N_CORES

    h, dk = _inv_kernel(np.asarray(inhibition_filter, np.float32), c)

    # windowed-band sanity: one-sided tail beyond T must be small, h must fit
    # fp8 scaling comfortably; otherwise fall back to an exact host FFT.
    dist = np.minimum(np.arange(c), c - np.arange(c))
    tail = np.sqrt((h[dist > T] ** 2).sum() / 2.0)
    ok = (
        c == C
        and n % N_CORES == 0
        and hw % FCH == 0
        and tail < 1.2e-2
        and np.abs(h).max() < 4.0
        and np.abs(h).sum() < 16.0
    )
    if not ok:
        fx = np.fft.fft(x.astype(np.float64), axis=1)
        fk = np.fft.fft(dk)
        y = np.fft.ifft(fx / fk[None, :, None, None], axis=1).real
        return y.astype(np.float32), None

    amax = float(np.abs(x).max()) + 1e-30
    SX = _pow2(16.0 / amax)
    SW = _pow2(16.0 / (np.abs(h).max() + 1e-30))
    SC = _pow2(16.0 / (np.abs(h).sum() * amax + 1e-30))
    out_scale = SC / (SX * SW)

    # window weight matrix: lhsT[kr, i] = h[i + T - kr] (signed circular lag)
    kr = np.arange(WIN)[:, None]
    ii = np.arange(M)[None, :]
    wq8 = np.clip(h[(ii + T - kr) % c] * SW, -31.0, 31.0).astype(F8)

    # pack x: per core [128, NW*npc*hw] e3m4, block j = w*npc + b
    rows = (np.arange(NW)[:, None] * M - T + np.arange(WIN)[None, :]) % c
    x8 = np.clip(x.reshape(n, c, hw) * SX, -31.0, 31.0).astype(F8)
    xg = x8[:, rows, :]                      # [n, NW, WIN, hw]
    xg = xg.reshape(N_CORES, npc, NW, WIN, hw).transpose(0, 3, 2, 1, 4)
    xs = np.ascontiguousarray(xg.reshape(N_CORES, WIN, NW * npc * hw))

    key = (npc, hw, out_scale)
    if key not in _CACHE:
        _CACHE[key] = _build_program(npc, hw, out_scale)
    nc = _CACHE[key]

    in_maps = [{"x": xs[i], "wq": wq8} for i in range(N_CORES)]
    try:
        res = run_bass_kernel_spmd(nc, in_maps, list(range(N_CORES)), trace=trace)
    except Exception:
        _reset_device()
        res = run_bass_kernel_spmd(nc, in_maps, list(range(N_CORES)), trace=trace)

    # y8 [core][NW, M, npc*hw]: device wrote e3m4(SC * correction)
    y8 = np.stack([res.results[i]["y"] for i in range(N_CORES)])
    corr = y8.astype(np.float32) / SC
    corr = corr.reshape(N_CORES, NW, M, npc, hw).transpose(0, 3, 1, 2, 4)
    corr = corr.reshape(n, NW * M, hw)[:, :c, :]

    y = x.reshape(n, c, hw) + corr
    return y.reshape(n, c, hgt, wid).astype(np.float32, copy=False), res


# revision 5
# speedup vs baseline: 2.1728x; 1.1262x over previous
"""Trainium2 kernel for FFT-based converged inhibition along the channel axis.

The reference computes y = IFFT(FFT(x, axis=C) / FFT(delta - k_padded)).real,
i.e. a circular convolution of each channel fiber with the fixed length-C
kernel g = IFFT(1/FFT(delta - k)).  Writing h = g - delta, the output is
y = x + h (*) x where the correction h (*) x is SMALL (||h||_2 ~ 0.14 for
this damping) and h decays fast away from lag 0.

Device strategy (8 NeuronCores, data-parallel over batch):
  - the device computes ONLY the correction c = h (*) x in fp8 (float8e3,
    4 mantissa bits); the host adds y = x + c in fp32.  This cuts HBM
    traffic per element from 8 B (fp32 in+out) to ~2.3 B, and the kernel
    is DMA-roofline bound.
  - channel axis split into NW=5 output windows of M=104; window w reads
    input rows [104w-12, 104w+115] (128 rows incl +-12 halo, mod C) so a
    single K=128 matmul per (window, column chunk) produces 104 output
    channels with the full h restricted to the window (only window-edge
    outputs see one-sided tap truncation; measured rel-err ~7e-3 vs the
    2e-2 budget on white-noise activations).
  - the window weight matrix lhsT[kr, i] = h[i + 12 - kr] is the same for
    every window -> one [128, 104] stationary tile, one LDWEIGHTS total.
  - scales: x is sent as e3m4(x * SX), weights as e3m4(h * SW), PSUM holds
    SX*SW*c, and the PSUM->SBUF copy applies SC/(SX*SW) and casts to e3m4;
    the host divides by SC.  All scales are powers of two chosen at run
    time from max|x| and the h norms, so they are exact.
"""

import numpy as np
import ml_dtypes

import concourse.bass as bass
import concourse.tile as tile
from concourse import bacc, mybir
from concourse.bass_utils import run_bass_kernel_spmd

N_CORES = 8
C = 512          # channels (FFT axis)
NW = 5           # output windows along C
M = 104          # output channels per window (NW * M = 520 >= C)
T = 12           # one-sided halo: window w reads rows [M*w - T, M*w - T + 127]
WIN = 128        # input rows per window
FCH = 448        # matmul moving free-dim chunk (3136 = 7 * 448, one PSUM bank)

_CACHE = {}

F8 = ml_dtypes.float8_e3m4  # TRN FP8_EXP3 (e3m4), max +-31


def _build_program(npc: int, hw: int, out_scale: float):
    """Per-core SPMD program: c[w] = (h-window) @ x[w] for NW windows."""
    nfc = hw // FCH
    assert nfc * FCH == hw
    nb = NW * npc  # input blocks (window, batch)
    nc = bacc.Bacc(
        "TRN2", target_bir_lowering=False, debug=False, enable_asserts=False
    )
    x_d = nc.dram_tensor(
        "x", [128, nb * hw], mybir.dt.float8e3, kind="ExternalInput"
    ).ap()
    w_d = nc.dram_tensor(
        "wq", [128, M], mybir.dt.float8e3, kind="ExternalInput"
    ).ap()
    y_d = nc.dram_tensor(
        "y", [NW, M, npc * hw], mybir.dt.float8e3, kind="ExternalOutput"
    ).ap()

    nchunk = npc * nfc  # matmul chunks per window (pairs for copy)
    assert nchunk % 2 == 0

    with tile.TileContext(nc) as tc:
        with (
            tc.tile_pool(name="wq", bufs=1) as w_pool,
            tc.tile_pool(name="x", bufs=1) as x_pool,
            tc.tile_pool(name="ps", bufs=4, space="PSUM") as ps_pool,
            tc.tile_pool(name="out", bufs=1) as out_pool,
        ):
            # dummy ACT op: forces the activation table load during the DMA
            # lead-in instead of blocking the first real PSUM copy.
            dumb = w_pool.tile([1, 1], mybir.dt.float32, tag="dumb")
            nc.scalar.mul(dumb[:], dumb[:], 1.0)

            w_sb = w_pool.tile([128, M], mybir.dt.float8e3, tag="wq")
            nc.sync.dma_start(w_sb[:], w_d)

            # inputs: block 0 split small-first so the PE starts (and HAM
            # warms) early; remaining (window, batch) blocks whole.
            xt = []
            t0 = x_pool.tile([128, hw], mybir.dt.float8e3, tag="x0")
            nc.sync.dma_start(t0[:, 0:FCH], x_d[:, 0:FCH])
            mid = (hw - FCH) // 2 + FCH
            nc.sync.dma_start(t0[:, FCH:mid], x_d[:, FCH:mid])
            nc.sync.dma_start(t0[:, mid:hw], x_d[:, mid:hw])
            xt.append(t0)
            for j in range(1, nb):
                t = x_pool.tile([128, hw], mybir.dt.float8e3, tag=f"x{j}")
                nc.sync.dma_start(t[:], x_d[:, j * hw : (j + 1) * hw])
                xt.append(t)

            eng = 0
            for w in range(NW):
                o = out_pool.tile([M, npc * hw], mybir.dt.float8e3, tag=f"o{w}")
                for p in range(nchunk // 2):
                    # paired PSUM tile: 2 bank-aligned matmul outputs
                    ps = ps_pool.tile(
                        [M, 2, 512], mybir.dt.float32, tag="ps",
                        name=f"ps{w}_{p}",
                    )
                    for j in range(2):
                        m = 2 * p + j
                        b, f = divmod(m, nfc)
                        nc.tensor.matmul(
                            ps[:, j, 0:FCH],
                            w_sb[:],
                            xt[w * npc + b][:, f * FCH : (f + 1) * FCH],
                            start=True,
                            stop=True,
                        )
                    dst = o[:, 2 * p * FCH : (2 * p + 2) * FCH]
                    src = ps[:, :, 0:FCH]
                    if eng % 2 == 0:
                        nc.vector.tensor_scalar_mul(dst, src, out_scale)
                    else:
                        nc.scalar.mul(dst, src, out_scale)
                    eng += 1
                for b in range(npc):
                    nc.sync.dma_start(
                        y_d[w, 0:M, b * hw : (b + 1) * hw],
                        o[:, b * hw : (b + 1) * hw],
                    )

    # Hoist no-wait input DMA dispatches into the pre-barrier main block so
    # transfers start while other engines are still in the entry barrier.
    try:
        main_blk = nc.main_func.blocks[0]
        sp = mybir.EngineType.SP
        moved = None
        for blk in nc.main_func.blocks[1:]:
            cand = [
                i
                for i in blk.instructions
                if i.engine == sp
                and isinstance(i, mybir.InstDMACopy)
                and not (i.sync_info and i.sync_info.on_wait)
            ]
            if cand:
                moved = cand[:8]
                for i in moved:
                    blk.instructions.remove(i)
                break
        if moved:
            pos = next(
                idx
                for idx, i in enumerate(main_blk.instructions)
                if i.engine == sp and isinstance(i, mybir.InstDrain)
            )
            main_blk.instructions[pos:pos] = moved
    except Exception:
        pass

    # Strip unused const-tile memsets from the preamble (they pull the gpsimd
    # ucode load into the critical entry barrier).
    for blk in nc.main_func.blocks:
        blk.instructions[:] = [
            inst
            for inst in blk.instructions
            if not (
                isinstance(inst, mybir.InstMemset)
                and inst.outs
                and "const-" in str(inst.outs[0])
            )
        ]
    nc.compile()
    return nc


def _inv_kernel(inhibition_filter: np.ndarray, c: int):
    """h = IFFT(1/FFT(delta - pad_roll(k))) - delta in float64."""
    scope = inhibition_filter.shape[0]
    k = np.zeros(c, np.float64)
    k[:scope] = inhibition_filter.astype(np.float64)
    k = np.roll(k, -(scope // 2))
    delta = np.zeros(c, np.float64)
    delta[0] = 1.0
    g = np.fft.ifft(1.0 / np.fft.fft(delta - k)).real
    return g - delta, delta - k


def _pow2(v: float) -> float:
    return float(2.0 ** np.floor(np.log2(v)))


def _reset_device():
    """Recover a wedged NeuronCore via axon."""
    try:
        import ctypes

        import jax

        jax.devices()
        lib = ctypes.CDLL("/opt/axon/libaxon_pjrt.so")
        if hasattr(lib, "axon_reset"):
            lib.axon_reset.restype = ctypes.c_int64
            lib.axon_reset()
    except Exception:
        pass


def kernel(activations: np.ndarray, inhibition_filter: np.ndarray) -> np.ndarray:
    return _run(activations, inhibition_filter, trace=False)[0]


def _run(activations, inhibition_filter, trace=False):
    x = np.ascontiguousarray(activations, dtype=np.float32)
    n, c, hgt, wid = x.shape
    hw = hgt * wid
    npc = n // N_CORES

    h, dk = _inv_kernel(np.asarray(inhibition_filter, np.float32), c)

    # windowed-band sanity: one-sided tail beyond T must be small, h must fit
    # fp8 scaling comfortably; otherwise fall back to an exact host FFT.
    dist = np.minimum(np.arange(c), c - np.arange(c))
    tail = np.sqrt((h[dist > T] ** 2).sum() / 2.0)
    ok = (
        c == C
        and n % N_CORES == 0
        and hw % FCH == 0
        and tail < 1.2e-2
        and np.abs(h).max() < 4.0
        and np.abs(h).sum() < 16.0
    )
    if not ok:
        fx = np.fft.fft(x.astype(np.float64), axis=1)
        fk = np.fft.fft(dk)
        y = np.fft.ifft(fx / fk[None, :, None, None], axis=1).real
        return y.astype(np.float32), None

    amax = float(np.abs(x).max()) + 1e-30
    SX = _pow2(16.0 / amax)
    SW = _pow2(16.0 / (np.abs(h).max() + 1e-30))
    SC = _pow2(16.0 / (np.abs(h).sum() * amax + 1e-30))
    out_scale = SC / (SX * SW)

    # window weight matrix: lhsT[kr, i] = h[i + T - kr] (signed circular lag)
    kr = np.arange(WIN)[:, None]
    ii = np.arange(M)[None, :]
    wq8 = np.clip(h[(ii + T - kr) % c] * SW, -31.0, 31.0).astype(F8)

    # pack x: per core [128, NW*npc*hw] e3m4, block j = w*npc + b
    rows = (np.arange(NW)[:, None] * M - T + np.arange(WIN)[None, :]) % c
    x8 = np.clip(x.reshape(n, c, hw) * SX, -31.0, 31.0).astype(F8)
    xg = x8[:, rows, :]                      # [n, NW, WIN, hw]
    xg = xg.reshape(N_CORES, npc, NW, WIN, hw).transpose(0, 3, 2, 1, 4)
    xs = np.ascontiguousarray(xg.reshape(N_CORES, WIN, NW * npc * hw))

    key = (npc, hw, out_scale)
    if key not in _CACHE:
        _CACHE[key] = _build_program(npc, hw, out_scale)
    nc = _CACHE[key]

    in_maps = [{"x": xs[i], "wq": wq8} for i in range(N_CORES)]
    try:
        res = run_bass_kernel_spmd(nc, in_maps, list(range(N_CORES)), trace=trace)
    except Exception:
        _reset_device()
        res = run_bass_kernel_spmd(nc, in_maps, list(range(N_CORES)), trace=trace)

    # y8 [core][NW, M, npc*hw]: device wrote e3m4(SC * correction)
    y8 = np.stack([res.results[i]["y"] for i in range(N_CORES)])
    corr = y8.astype(np.float32) / SC
    corr = corr.reshape(N_CORES, NW, M, npc, hw).transpose(0, 3, 1, 2, 4)
    corr = corr.reshape(n, NW * M, hw)[:, :c, :]

    y = x.reshape(n, c, hw) + corr
    return y.reshape(n, c, hgt, wid).astype(np.float32, copy=False), res
